# revision 53
# baseline (speedup 1.0000x reference)
"""Multi-head attention (B=4, T=S=2048, E=1024, H=16, D=64) on 8 TRN2 NeuronCores.

Sharding: core c handles batch b=c//2 and head-group g=c%2 (8 of 16 heads).
Each core computes its 8 heads' attention plus the matching column-slice of
the output projection, producing a partial [T, E] f32 output. Host sums the
two partials per batch and adds bo.

On-chip dataflow (fp32 PSUM accumulation throughout):
  qT[d,t] = WqT.T @ queryT       (d-major projections, per 128-dim head pair;
                                  fp8 DoubleRow with hi+lo error compensation:
                                  xh*Wh + xl*Wh + xh*Wl at 0.5 cycles/row)
  kT[d,t] likewise; v[s,d] natural via value.T as the stationary operand
  S.T[s,t] = kT_h.T @ qT_h       (two heads row-packed in the 128-row PE array)
  expS.T   = exp(S.T * 1/8)      (ScalarE, PSUM -> SBUF bf16)
  [O;den]  = expS_blk.T @ [v_h|1]  (exp block stationary, narrow 65-col v
                                    moving -> O in [t,d] layout + per-t dens)
  Onorm_td = O * (1/den)         (den is per-PARTITION here: one cheap DVE
                                  tensor_tensor, no cross-partition bcast)
  OnormT   = XBAR DMA-transpose of Onorm_td -> d-major   (no PE/PSUM cost)
  partial  = OnormT.T @ WoSlice  (accumulate over the core's 4 head pairs)

The [t,d]-output PV orientation keeps the PE's moving operand narrow (65
cols vs 512), halving PV time; denominators ride along as a 65th moving
column of ones, and the d-major layout the out-projection needs is restored
by the DMA crossbar instead of the PE.

Activations and projection weights arrive as host-packed fp8 hi/lo pair
layouts (pre-scaled into e4m3's normal range; the 2^-15 product scale is
folded into the f32 bias-add), streamed in few large DMAs issued a stage
ahead of their consumers; projections are emitted as quarter-granular
accumulation groups so their PSUM residency is short and their PE work
spreads smoothly across the score slots.

Emission is software-pipelined: stage s=(pair, t-quarter) in pair-major
order; each stage's 16 score-tile slots interleave the previous stage's PV
accumulation plus spread-out projection / v-projection / out-projection
work, keeping ScalarE (the exp bottleneck) continuously fed.
"""

from contextlib import ExitStack

import numpy as np
import ml_dtypes

B, T, S, E = 4, 2048, 2048, 1024
H, D = 16, 64
DC = 512          # dims per core (8 heads x 64)
NP = 4            # head pairs per core
NS = S // 128     # 16 s-tiles
NQ = 4            # t-quarters of 512

_BF16 = ml_dtypes.bfloat16
_F8 = ml_dtypes.float8_e4m3fn


def _pair_planes(xT, scale):
    """Split into fp8 hi+lo planes after pre-scaling into e4m3's normal
    range (the raw values sit near the subnormal floor, where the lo plane
    would quantize away); the matmul results carry the combined scale,
    removed in the f32 bias-add."""
    xs = xT.astype(np.float32) * scale
    hi = xs.astype(_F8)
    lo = (xs - hi.astype(np.float32)).astype(_F8)
    return hi, lo


def _x8_layout(xT):
    """[E, 2048] f32 -> [128, 32768] fp8 hi/lo chunk-pair layout: col =
    q*8192 + P*2048 + c2*1024 + pl*512 + tau, row p, where the source row
    is e = (2P + c2)*128 + p and t = q*512 + tau."""
    hi, lo = _pair_planes(xT, 32.0)
    A = np.stack([hi, lo]).reshape(2, 4, 2, 128, 4, 512)
    A = A.transpose(3, 4, 1, 2, 0, 5)
    return np.ascontiguousarray(A.reshape(128, 4 * 8192))


def _w8_layout(WT):
    """[E, DC] f32 -> [128, 8192] fp8: col = P*2048 + c2*1024 + pl*512 + d."""
    hi, lo = _pair_planes(WT, 1024.0)
    A = np.stack([hi, lo]).reshape(2, 4, 2, 128, 512)
    A = A.transpose(3, 1, 2, 0, 4)
    return np.ascontiguousarray(A.reshape(128, 8192))

_cached = None


def _build(repeats=1):
    import concourse.bass as bass
    import concourse.mybir as mybir
    import concourse.tile as tile
    from concourse import bacc

    f32 = mybir.dt.float32
    bf16 = mybir.dt.bfloat16
    i32 = mybir.dt.int32
    AF = mybir.ActivationFunctionType
    # Schraudolph exp constants (y in units of raw score; exp(y/8) =
    # bitcast_f32(int32(y * 2^23/(8 ln2) + (127*2^23 - C)))), C tuned for
    # min RMS relative error under truncating conversion.
    SCH_A = float(np.float32(2 ** 23 / np.log(2) / 8.0))
    SCH_B = float(np.float32(127 * 2 ** 23 - 488000))

    nc = bacc.Bacc("TRN2", target_bir_lowering=False)

    f8 = mybir.dt.float8e4
    # fp8 hi/lo pair layouts (host-packed to match SBUF tiles exactly):
    # activations [128, 4 quarters x 8192] with quarter cols
    # P*2048 + c2*1024 + pl*512 + tau  (chunk-pair P, chunk c2, hi/lo pl);
    # weights [128, 8192] with cols P*2048 + c2*1024 + pl*512 + d.
    q8_d = nc.dram_tensor("q8", [128, 4 * 8192], f8, kind="ExternalInput")
    k8_d = nc.dram_tensor("k8", [128, 4 * 8192], f8, kind="ExternalInput")
    v8_d = nc.dram_tensor("v8", [128, 4 * 8192], f8, kind="ExternalInput")
    wq8_d = nc.dram_tensor("wq8", [128, 8192], f8, kind="ExternalInput")
    wk8_d = nc.dram_tensor("wk8", [128, 8192], f8, kind="ExternalInput")
    wv8_d = nc.dram_tensor("wv8", [128, 8192], f8, kind="ExternalInput")
    WoS_d = nc.dram_tensor("WoS", [DC, E], bf16, kind="ExternalInput")
    bq_d = nc.dram_tensor("bq", [128, NP], f32, kind="ExternalInput")
    bk_d = nc.dram_tensor("bk", [128, NP], f32, kind="ExternalInput")
    bv_d = nc.dram_tensor("bv", [1, DC], f32, kind="ExternalInput")
    out_d = nc.dram_tensor("out", [T, E], bf16, kind="ExternalOutput")

    with tile.TileContext(nc) as tc, ExitStack() as ctx:
        persist = ctx.enter_context(tc.tile_pool(name="persist", bufs=1))
        psc = ctx.enter_context(tc.tile_pool(name="psc", bufs=2, space="PSUM"))
        ppv = ctx.enter_context(tc.tile_pool(name="ppv", bufs=1, space="PSUM"))
        pmx = ctx.enter_context(tc.tile_pool(name="pmx", bufs=2, space="PSUM"))
        expool = ctx.enter_context(tc.tile_pool(name="expool", bufs=22))
        small = ctx.enter_context(tc.tile_pool(name="small", bufs=2))
        ocp_pool = ctx.enter_context(tc.tile_pool(name="ocp", bufs=2))
        xin = ctx.enter_context(tc.tile_pool(name="xin", bufs=14))
        wpool = ctx.enter_context(tc.tile_pool(name="wts", bufs=3))

        # ---- persistent SBUF tiles ----
        qTs = [persist.tile([128, T], bf16, tag=f"qT{p}", name=f"qT{p}") for p in range(NP)]
        kTs = [persist.tile([128, S], bf16, tag=f"kT{p}", name=f"kT{p}") for p in range(NP)]
        vaug = [persist.tile([128, 8 * 65], bf16, tag=f"va{st}", name=f"va{st}") for st in range(NS)]
        WoSs = [persist.tile([128, E], bf16, tag=f"wo{p}", name=f"wo{p}") for p in range(NP)]
        Onorm = [persist.tile([128, T], bf16, tag=f"on{p}", name=f"on{p}") for p in range(NP)]
        bq_sb = persist.tile([128, NP], f32, tag="bq", name="bq_sb")
        bk_sb = persist.tile([128, NP], f32, tag="bk", name="bk_sb")
        bv_sb = persist.tile([128, DC], f32, tag="bv", name="bv_sb")

        def load_biases():
            nc.sync.dma_start(out=bq_sb, in_=bq_d[:, :])
            nc.sync.dma_start(out=bk_sb, in_=bk_d[:, :])
            bv_ap = bv_d[:, :]
            bv_bcast_ap = bass.AP(
                tensor=bv_ap.tensor,
                offset=bv_ap.offset,
                ap=[[0, 128], bv_ap.ap[-1]],
            )
            nc.sync.dma_start(out=bv_sb, in_=bv_bcast_ap)
        for st in range(NS):
            va3 = vaug[st].rearrange("p (h x) -> p h x", x=65)
            nc.vector.memset(va3[:, :, 64:65], 1.0)

        def load_w(dram, split=False):
            """One [128, 8192] fp8 tile holding the hi/lo chunk-pair layout
            of one projection weight (single DMA). With split=True, pair-0's
            128-col d-slices (the only ones the startup stages touch) load
            first and a thunk for the rest is returned."""
            wt = wpool.tile([128, 8192], f8, tag="w", name="wt")
            w4 = wt.rearrange("p (b d) -> p b d", d=512)
            s4 = dram[:, :].rearrange("p (b d) -> p b d", d=512)
            if not split:
                nc.sync.dma_start(out=wt, in_=dram[:, :])
                return wt
            nc.sync.dma_start(out=w4[:, :, 0:128], in_=s4[:, :, 0:128])
            def rest():
                nc.sync.dma_start(out=w4[:, :, 128:512], in_=s4[:, :, 128:512])
            return wt, rest

        def pair_ap(t, off, n):
            """AP selecting the two 1024-stride k-tile blocks at off: free
            dims [2, n] as DoubleRow wants."""
            a = t[:, off:off + n]
            return bass.AP(tensor=a.tensor, offset=a.offset,
                           ap=[a.ap[0], [1024, 2], [1, n]])

        def xq_dma(x_dram, q):
            """Two [128, 2x2048] fp8 tiles (chunk-pairs 0-1 and 2-3) of one
            512-col quarter of an activation input (two contiguous DMAs)."""
            xh = []
            for hv in range(2):
                xt = xin.tile([128, 4096], f8, tag="xin", name="xin")
                nc.sync.dma_start(
                    out=xt,
                    in_=x_dram[:, q * 8192 + hv * 4096:
                               q * 8192 + (hv + 1) * 4096],
                )
                xh.append(xt)
            return xh

        def dma_box(x_dram, q):
            """Thunk that issues xq_dma when called (placed a stage ahead
            of the matmul consumers), exposing the tiles via .get()."""
            box = []
            def run():
                box.extend(xq_dma(x_dram, q))
            run.get = lambda: box
            return run

        def proj_quarter(p, wt, get_xq, dst, bias_sb, q):
            """Work thunks for one projection output quarter: a 2-thunk,
            12-matmul fp8 DoubleRow accumulation group computing the
            error-compensated xh*Wh + xl*Wh + xh*Wl over all 4 chunk-pairs,
            then bias. get_xq resolves the quarter's input tiles at emission
            time (they were DMA'd a stage earlier)."""
            ps_box = []
            DR = mybir.MatmulPerfMode.DoubleRow

            def half(P0, first, last, ps_box=ps_box):
                xh = get_xq()
                if first:
                    ps_box.append(pmx.tile([128, 512], f32, tag="mx", name="mx_ps"))
                for P in (P0, P0 + 1):
                    loc = (P % 2) * 2048
                    wh = P * 2048 + p * 128
                    for i, (s_off, m_off) in enumerate(
                            ((wh, loc), (wh, loc + 512), (wh + 512, loc))):
                        nc.tensor.matmul(
                            ps_box[0],
                            pair_ap(wt, s_off, 128),
                            pair_ap(xh[P // 2], m_off, 512),
                            start=(first and P == P0 and i == 0),
                            stop=(last and P == P0 + 1 and i == 2),
                            perf_mode=DR,
                        )

            def mm_lo():
                half(0, True, False)

            def mm_hi(ps_box=ps_box):
                half(2, False, True)
                nc.vector.tensor_scalar(
                    out=dst[:, q * 512:(q + 1) * 512],
                    in0=ps_box[0], scalar1=1.0 / (32.0 * 1024.0),
                    scalar2=bias_sb[:, p:p + 1],
                    op0=mybir.AluOpType.mult, op1=mybir.AluOpType.add)
                ps_box.clear()

            return [mm_lo, mm_hi]

        def vproj_units(wt, dh, get_vq, stis):
            """V-projection work for head-quad dh: per unit the
            error-compensated fp8 DoubleRow full-E contraction for one
            128-row s-tile + bias into the augmented-v layout. get_vq(st//4)
            resolves the s-quarter's input tiles."""
            thunks = []
            DR = mybir.MatmulPerfMode.DoubleRow
            for st in stis:
                def vst(st=st):
                    vh = get_vq(st // 4)
                    si = st % 4
                    ps = pmx.tile([128, 512], f32, tag="mx", name="mx_ps")
                    for P in range(4):
                        loc = (P % 2) * 2048 + si * 128
                        wo = P * 2048 + dh * 256
                        for i, (s_off, m_off) in enumerate(
                                ((loc, wo), (loc + 512, wo), (loc, wo + 512))):
                            nc.tensor.matmul(
                                ps[:, 0:256],
                                pair_ap(vh[P // 2], s_off, 128),
                                pair_ap(wt, m_off, 256),
                                start=(P == 0 and i == 0),
                                stop=(P == 3 and i == 2),
                                perf_mode=DR,
                            )
                    va3 = vaug[st].rearrange("p (h x) -> p h x", x=65)
                    nc.vector.scalar_tensor_tensor(
                        out=va3[:, dh * 4:(dh + 1) * 4, 0:64],
                        in0=ps[:, 0:256].rearrange("p (h x) -> p h x", x=64),
                        scalar=1.0 / (32.0 * 1024.0),
                        in1=bv_sb[:, dh * 256:(dh + 1) * 256].rearrange(
                            "p (h x) -> p h x", x=64),
                        op0=mybir.AluOpType.mult, op1=mybir.AluOpType.add,
                    )
                thunks.append(vst)
            return thunks

        def outproj_thunks(tq):
            """8 [t-block, 512]-column units. The first two are split into
            open-with-pairs-0-2 / append-pair-3 thunk pairs: pair 3's Onorm
            quarter arrives via the freshest transpose, so priming both PSUM
            groups first hides that latency from the in-order PE queue."""
            units = [(tt, c) for tt in range(tq * 4, tq * 4 + 4)
                     for c in range(2)]
            boxes = {}

            def emit_mms(ps, tt, c, prange, start, stop):
                for p in prange:
                    nc.tensor.matmul(
                        ps,
                        Onorm[p][:, tt * 128:(tt + 1) * 128],
                        WoSs[p][:, c * 512:(c + 1) * 512],
                        start=(start and p == prange[0]),
                        stop=(stop and p == prange[-1]),
                    )

            def store(ps, tt, c):
                oc = ocp_pool.tile([128, 512], bf16, tag="ocp", name="oc",
                                   bufs=3)
                nc.vector.tensor_copy(oc, ps)
                nc.sync.dma_start(
                    out=out_d[tt * 128:(tt + 1) * 128,
                              c * 512:(c + 1) * 512],
                    in_=oc)

            def pre(i):
                def f(i=i):
                    tt, c = units[i]
                    ps = pmx.tile([128, 512], f32, tag="mx", name="mx_ps")
                    boxes[i] = ps
                    emit_mms(ps, tt, c, (0, 1, 2), True, False)
                return f

            def fin(i):
                def f(i=i):
                    tt, c = units[i]
                    ps = boxes.pop(i)
                    emit_mms(ps, tt, c, (3,), False, True)
                    store(ps, tt, c)
                return f

            def unit(i):
                def f(i=i):
                    tt, c = units[i]
                    ps = pmx.tile([128, 512], f32, tag="mx", name="mx_ps")
                    emit_mms(ps, tt, c, (0, 1, 2, 3), True, True)
                    store(ps, tt, c)
                return f

            return ([pre(0), pre(1), fin(0), fin(1)]
                    + [unit(i) for i in range(2, 8)])

        def outproj_tail(tq, tts=None):
            # tail variant: "sc" psum tiles are free once scoring has ended,
            # so use wide [128,1024] units to avoid mx-slot serialization
            def run():
                for tt in (tts if tts is not None
                           else range(tq * 4, tq * 4 + 4)):
                    op_ps = psc.tile([128, 1024], f32, tag="sc", name="sc_ps")
                    for c in range(2):
                        for p in range(NP):
                            nc.tensor.matmul(
                                op_ps[:, c * 512:(c + 1) * 512],
                                Onorm[p][:, tt * 128:(tt + 1) * 128],
                                WoSs[p][:, c * 512:(c + 1) * 512],
                                start=(p == 0),
                                stop=(p == 3),
                            )
                    oc = ocp_pool.tile([128, 1024], bf16, tag="ocpw", name="ocw")
                    nc.vector.tensor_copy(oc, op_ps)
                    nc.sync.dma_start(out=out_d[tt * 128:(tt + 1) * 128, :],
                                      in_=oc)
            return run

        class PrevStage:
            def __init__(self, p, tq, exs):
                self.p, self.tq, self.exs = p, tq, exs
                self.pv = None      # two [128, 4*65] psum tiles (4 units each)

        def emit_pv_slot(prev, st):
            """8 accumulating matmuls: O_td[t, d]/den for each (tb, h) unit,
            with exp (s x t block) stationary and the 65-col augmented v
            moving. Unit g = tb*2 + h lives at cols (g%4)*65 of pv tile g//4;
            one start per PSUM bank (unit 0 of each tile, which pending-zeroes
            the whole bank), one stop (unit 3)."""
            if st == 0:
                prev.pv = [
                    ppv.tile([128, 260], f32, tag="pvA", name="pvA",
                             padded_shape=[128, 512]),
                    ppv.tile([128, 260], f32, tag="pvB", name="pvB",
                             padded_shape=[128, 512]),
                ]
            for g in range(8):
                tb, h = g // 2, g % 2
                pvt = prev.pv[g // 4]
                u = g % 4
                hidx = 2 * prev.p + h
                nc.tensor.matmul(
                    pvt[:, u * 65:(u + 1) * 65],
                    prev.exs[st][:, h * 512 + tb * 128:h * 512 + (tb + 1) * 128],
                    vaug[st][:, hidx * 65:hidx * 65 + 65],
                    start=(st == 0 and u == 0),
                    stop=(st == 15 and u == 3),
                )

        def emit_normalize_half(prev, i, rc, otd):
            """Normalize + transpose for one pv tile (4 units = 2 t-blocks):
            used at the tail so the first out-proj blocks start while the
            second half still normalizes."""
            pv3 = prev.pv[i].rearrange("p (u x) -> p u x", x=65)
            nc.vector.reciprocal(rc[:, i * 4:(i + 1) * 4], pv3[:, :, 64:65])
            rc_b = bass.AP(
                tensor=rc.tensor,
                offset=rc[:, i * 4:(i + 1) * 4].offset,
                ap=[rc.ap[0], [1, 4], [0, 64]],
            )
            nc.vector.tensor_mul(
                otd[:, i * 256:(i + 1) * 256].rearrange(
                    "p (u x) -> p u x", x=64),
                pv3[:, :, 0:64],
                rc_b,
            )
            t0 = prev.tq * 512 + i * 256
            nc.sync.dma_start_transpose(
                out=Onorm[prev.p][:, t0:t0 + 256].rearrange(
                    "p (b t) -> p b t", t=128),
                in_=otd[:, i * 256:(i + 1) * 256],
            )

        def emit_normalize(prev):
            """den sits on the same partition as its O row: reciprocal of the
            65th column of each unit, one tensor_tensor mul per pv tile (rc
            broadcast along d via a stride-0 AP) into a [t, d] bf16 tile, then
            one XBAR DMA transpose restoring the d-major layout out-proj
            wants (out 3D => per-128-block transpose)."""
            rc = small.tile([128, 8], f32, tag="rc", name="rc")
            otd = small.tile([128, 512], bf16, tag="otd", name="otd")
            for i in range(2):
                pv3 = prev.pv[i].rearrange("p (u x) -> p u x", x=65)
                nc.vector.reciprocal(rc[:, i * 4:(i + 1) * 4], pv3[:, :, 64:65])
            for i in range(2):
                pv3 = prev.pv[i].rearrange("p (u x) -> p u x", x=65)
                rc_b = bass.AP(
                    tensor=rc.tensor,
                    offset=rc[:, i * 4:(i + 1) * 4].offset,
                    ap=[rc.ap[0], [1, 4], [0, 64]],
                )
                nc.vector.tensor_mul(
                    otd[:, i * 256:(i + 1) * 256].rearrange(
                        "p (u x) -> p u x", x=64),
                    pv3[:, :, 0:64],
                    rc_b,
                )
            t0 = prev.tq * 512
            nc.sync.dma_start_transpose(
                out=Onorm[prev.p][:, t0:t0 + 512].rearrange(
                    "p (b t) -> p b t", t=128),
                in_=otd,
            )
            prev.pv = None

        DVE_EXP_ST = (4, 9, 14)

        def emit_stage(p, tq, prev, extras, dl=16, sl0=0, dve_exp=True):
            """16 score slots for (p, tq); interleave prev stage's PV and
            the extra thunks (emitted between slots sl0..sl0+dl); returns
            this stage's record. Three slots per stage compute exp on the
            DVE (Schraudolph bit-trick, ~1.8% rms) to keep ScalarE off the
            critical path; the rest use the exact ScalarE table exp."""
            t0 = tq * 512
            exs = []
            n_ex = len(extras)
            taken = 0
            for st in range(NS):
                # scores lead the slot: exp(st) can then start while the PE
                # works through this slot's PV and extras, keeping ScalarE
                # saturated; the score psum's WAR wait (on exp st-2) is
                # covered by the previous slot's trailing work.
                sc_ps = psc.tile([128, 1024], f32, tag="sc", name="sc_ps")
                nc.tensor.matmul(
                    sc_ps[:, 0:512],
                    kTs[p][0:64, st * 128:(st + 1) * 128],
                    qTs[p][0:64, t0:t0 + 512],
                    start=True, stop=True,
                    tile_position=(0, 0),
                )
                nc.tensor.matmul(
                    sc_ps[:, 512:1024],
                    kTs[p][64:128, st * 128:(st + 1) * 128],
                    qTs[p][64:128, t0:t0 + 512],
                    start=True, stop=True,
                    tile_position=(64, 0),
                )
                ex = expool.tile([128, 1024], bf16, tag="ex", name="ex")
                if dve_exp and st in DVE_EXP_ST:
                    sch = small.tile([128, 1024], i32, tag="sch", name="sch",
                                     bufs=1)
                    nc.vector.tensor_scalar(
                        out=sch, in0=sc_ps, scalar1=SCH_A, scalar2=SCH_B,
                        op0=mybir.AluOpType.mult, op1=mybir.AluOpType.add)
                    nc.vector.tensor_copy(ex, sch.bitcast(f32))
                else:
                    nc.scalar.activation(ex, sc_ps, AF.Exp, scale=0.125)
                exs.append(ex)
                if prev is not None:
                    emit_pv_slot(prev, st)
                if st >= sl0:
                    prog = min(st - sl0 + 1, dl)
                    want = (n_ex * prog + dl - 1) // dl
                    while taken < want:
                        extras[taken]()
                        taken += 1
            while taken < n_ex:
                extras[taken]()
                taken += 1
            if prev is not None:
                rc = small.tile([128, 8], f32, tag="rc", name="rc")
                otd = small.tile([128, 512], bf16, tag="otd", name="otd")
                emit_normalize_half(prev, 0, rc, otd)
                emit_normalize_half(prev, 1, rc, otd)
                prev.pv = None
            return PrevStage(p, tq, exs)

        # ---- emission ----
        for _rep in range(repeats):
            # startup: stream in pair-0's q/k projections at quarter
            # granularity — the first scores need only q quarter-0 and k
            # quarter-0; later k quarters land just before their score
            # slots. Only pair-0's weight slices load up-front.
            wq, wq_rest = load_w(wq8_d, split=True)
            xq_q0 = xq_dma(q8_d, 0)
            wk, wk_rest = load_w(wk8_d, split=True)
            xk_q0 = xq_dma(k8_d, 0)
            load_biases()
            xk_q1 = xq_dma(k8_d, 1)
            for th in proj_quarter(0, wq, lambda: xq_q0, qTs[0], bq_sb, 0):
                th()
            for th in proj_quarter(0, wk, lambda: xk_q0, kTs[0], bk_sb, 0):
                th()
            xk_q2 = xq_dma(k8_d, 2)
            xq_q1 = xq_dma(q8_d, 1)            # q quarter 1: stage 1's scores
            xk_q3 = xq_dma(k8_d, 3)
            for th in proj_quarter(0, wk, lambda: xk_q1, kTs[0], bk_sb, 1):
                th()
            wk_rest()
            wq_rest()
            wv = load_w(wv8_d)
            for p in range(NP):
                nc.sync.dma_start(out=WoSs[p], in_=WoS_d[p * 128:(p + 1) * 128, :])

            extras = {}

            def add(sg, ths):
                extras[sg] = extras.get(sg, []) + ths

            # stage 0: k quarters 2/3 (this stage's slots 8-15), q quarter 1
            # (stage 1's scores), vaug st0-7; stage 1: q quarters 2/3
            # (stage 2's scores), vaug st8-15.
            xv_q0_t = dma_box(v8_d, 0)
            xv_q1_t = dma_box(v8_d, 1)
            xv_q2_t = dma_box(v8_d, 2)
            xv_q3_t = dma_box(v8_d, 3)
            xq_q2_t = dma_box(q8_d, 2)
            xq_q3_t = dma_box(q8_d, 3)

            def get_vA(qv):
                return {0: xv_q0_t.get, 1: xv_q1_t.get,
                        2: xv_q2_t.get, 3: xv_q3_t.get}[qv]()

            add(0, [xv_q0_t, xv_q1_t])
            add(0, proj_quarter(0, wk, lambda: xk_q2, kTs[0], bk_sb, 2))
            add(0, proj_quarter(0, wk, lambda: xk_q3, kTs[0], bk_sb, 3))
            add(0, proj_quarter(0, wq, lambda: xq_q1, qTs[0], bq_sb, 1))
            add(0, [xv_q2_t, xv_q3_t, xq_q2_t, xq_q3_t])
            add(0, vproj_units(wv, 0, get_vA, range(0, 8)))
            add(1, proj_quarter(0, wq, xq_q2_t.get, qTs[0], bq_sb, 2))
            add(1, proj_quarter(0, wq, xq_q3_t.get, qTs[0], bq_sb, 3))
            add(1, vproj_units(wv, 0, get_vA, range(8, 16)))

            # pairs 1-3 q/k at quarter granularity: k q0/q1 at stage 4p-2,
            # q q0/q1 at 4p-1, k q2/q3 at 4p (hard slot-8 deadline), q q2/q3
            # at 4p+1; each quarter's two DMAs issue one stage ahead.
            for p in range(1, NP):
                bx = {key: dma_box(d, q)
                      for key, d, q in (("k0", k8_d, 0), ("k1", k8_d, 1),
                                        ("k2", k8_d, 2), ("k3", k8_d, 3),
                                        ("q0", q8_d, 0), ("q1", q8_d, 1),
                                        ("q2", q8_d, 2), ("q3", q8_d, 3))}
                add(4 * p - 3, [bx["k0"], bx["k1"]])
                add(4 * p - 2, proj_quarter(p, wk, bx["k0"].get, kTs[p], bk_sb, 0)
                    + proj_quarter(p, wk, bx["k1"].get, kTs[p], bk_sb, 1)
                    + [bx["q0"], bx["q1"]])
                add(4 * p - 1, proj_quarter(p, wq, bx["q0"].get, qTs[p], bq_sb, 0)
                    + proj_quarter(p, wq, bx["q1"].get, qTs[p], bq_sb, 1)
                    + [bx["k2"], bx["k3"]])
                add(4 * p, proj_quarter(p, wk, bx["k2"].get, kTs[p], bk_sb, 2)
                    + proj_quarter(p, wk, bx["k3"].get, kTs[p], bk_sb, 3)
                    + [bx["q2"], bx["q3"]])
                add(4 * p + 1, proj_quarter(p, wq, bx["q2"].get, qTs[p], bq_sb, 2)
                    + proj_quarter(p, wq, bx["q3"].get, qTs[p], bq_sb, 3))

            # second head-quad of V (pairs 2-3, needed from stage 9's PV):
            # spread over lighter stages 2/3/5/6, s-quarter DMAs one stage
            # ahead (the vT data is re-fetched; keeping the vpA tiles alive
            # that long would overflow the input pool).
            vB = {qv: dma_box(v8_d, qv) for qv in range(4)}

            def get_vB(qv):
                return vB[qv].get()

            add(1, [vB[0]])
            add(2, vproj_units(wv, 1, get_vB, range(0, 4)) + [vB[1]])
            add(3, vproj_units(wv, 1, get_vB, range(4, 8)))
            add(4, [vB[2]])
            add(5, vproj_units(wv, 1, get_vB, range(8, 12)) + [vB[3]])
            add(6, vproj_units(wv, 1, get_vB, range(12, 16)))

            add(14, outproj_thunks(0))
            add(15, outproj_thunks(1))

            # DMA-issue thunks cost nothing on the PE: run them at each
            # stage's first slots so their transfers lead the consumers.
            for sg in extras:
                extras[sg].sort(key=lambda th: not hasattr(th, "get"))

            # pacing deadlines: K-half1 stages (4p) must finish extras by
            # slot 8 (their own scores need those kT columns); stages 0/1
            # feed vaug just-in-time; elsewhere spread smoothly. Stages
            # 14/15's out-proj starts at slot 2 (their Onorm quarter lands
            # via the DMA transpose issued at the previous stage's end).
            dls = {0: 16, 1: 14, 4: 8, 8: 8, 12: 8, 14: 15, 15: 15}
            sl0s = {14: 4, 15: 4}
            prev = None
            for s in range(16):
                p, tq = s // 4, s % 4
                prev = emit_stage(p, tq, prev, extras.get(s, []),
                                  dl=dls.get(s, 16), sl0=sl0s.get(s, 0),
                                  dve_exp=False)

            # tail: PV of the last stage with out-proj(t2) interleaved
            # (its Onorm slices land with the transpose at the end of stage
            # 15), then the final normalize+transpose and out-proj(t3)
            op2 = outproj_thunks(2)
            taken = 0
            for st in range(NS):
                if st >= 2:
                    want = (len(op2) * (st - 1) + 13) // 14
                    while taken < want:
                        op2[taken]()
                        taken += 1
                emit_pv_slot(prev, st)
            # pre-accumulate pairs 0-2 of out-proj(q3) for t-blocks 12/13
            # (their Onorm quarters landed at stages 4/8/12) while pair 3's
            # normalize -> XBAR-transpose chain drains; append only the p3
            # matmuls once it lands.
            pre = []
            for tt in (12, 13):
                op_ps = psc.tile([128, 1024], f32, tag="sc", name="sc_ps")
                for c in range(2):
                    for p in range(3):
                        nc.tensor.matmul(
                            op_ps[:, c * 512:(c + 1) * 512],
                            Onorm[p][:, tt * 128:(tt + 1) * 128],
                            WoSs[p][:, c * 512:(c + 1) * 512],
                            start=(p == 0), stop=False,
                        )
                pre.append((tt, op_ps))
            pre2 = []
            for c in range(2):       # t-block 14's halves prime the mx slots
                ps = pmx.tile([128, 512], f32, tag="mx", name="mx_ps")
                for p in range(3):
                    nc.tensor.matmul(
                        ps, Onorm[p][:, 14 * 128:15 * 128],
                        WoSs[p][:, c * 512:(c + 1) * 512],
                        start=(p == 0), stop=False,
                    )
                pre2.append(ps)
            rc = small.tile([128, 8], f32, tag="rc", name="rc")
            otd = small.tile([128, 512], bf16, tag="otd", name="otd")
            emit_normalize_half(prev, 0, rc, otd)
            emit_normalize_half(prev, 1, rc, otd)
            for tt, op_ps in pre:
                for c in range(2):
                    nc.tensor.matmul(
                        op_ps[:, c * 512:(c + 1) * 512],
                        Onorm[3][:, tt * 128:(tt + 1) * 128],
                        WoSs[3][:, c * 512:(c + 1) * 512],
                        start=False, stop=(c == 1),
                    )
                oc = ocp_pool.tile([128, 1024], bf16, tag="ocpw", name="ocw")
                nc.vector.tensor_copy(oc, op_ps)
                nc.sync.dma_start(out=out_d[tt * 128:(tt + 1) * 128, :], in_=oc)
            for c in range(2):
                ps = pre2[c]
                nc.tensor.matmul(
                    ps, Onorm[3][:, 14 * 128:15 * 128],
                    WoSs[3][:, c * 512:(c + 1) * 512],
                    start=False, stop=True,
                )
                oc = ocp_pool.tile([128, 512], bf16, tag="ocp", name="oc",
                                   bufs=3)
                nc.vector.tensor_copy(oc, ps)
                nc.sync.dma_start(
                    out=out_d[14 * 128:15 * 128, c * 512:(c + 1) * 512],
                    in_=oc)
            outproj_tail(3, (15,))()
            prev.pv = None

    nc.compile()
    return nc


def _get_nc():
    global _cached
    if _cached is None:
        _cached = _build()
    return _cached


def _prep_core_inputs(c, query, key, value, Wq, Wk, Wv, Wo, bq, bk, bv,
                      _cache={}):
    b, g = c // 2, c % 2
    sl = slice(g * DC, (g + 1) * DC)
    key_ = (id(query), b)
    if key_ not in _cache:
        # both cores of a batch share the packed fp8 activations
        _cache.clear()
        _cache[key_] = {
            "q8": _x8_layout(query[b].T),
            "k8": _x8_layout(key[b].T),
            "v8": _x8_layout(value[b].T),
        }
    shared = _cache[key_]
    return {
        **shared,
        "wq8": _w8_layout(Wq[sl].T),
        "wk8": _w8_layout(Wk[sl].T),
        "wv8": _w8_layout(Wv[sl].T),
        "WoS": Wo[:, sl].T.astype(_BF16),
        "bq": np.ascontiguousarray(bq[sl].reshape(NP, 128).T),
        "bk": np.ascontiguousarray(bk[sl].reshape(NP, 128).T),
        "bv": np.ascontiguousarray(bv[sl].reshape(1, DC)),
    }


def kernel(**inputs):
    from concourse.bass_utils import run_bass_kernel_spmd

    args = {k: np.asarray(inputs[k], np.float32)
            for k in ("query", "key", "value", "Wq", "Wk", "Wv", "Wo",
                      "bq", "bk", "bv", "bo")}
    _prep_core_inputs.__defaults__[0].clear()
    nc = _get_nc()
    in_maps = [
        _prep_core_inputs(c, args["query"], args["key"], args["value"],
                          args["Wq"], args["Wk"], args["Wv"], args["Wo"],
                          args["bq"], args["bk"], args["bv"])
        for c in range(8)
    ]
    res = run_bass_kernel_spmd(nc, in_maps, core_ids=list(range(8)))
    outs = [r["out"] for r in res.results]
    final = np.empty((B, T, E), np.float32)
    for b in range(B):
        final[b] = (outs[2 * b].astype(np.float32)
                    + outs[2 * b + 1].astype(np.float32)
                    + args["bo"][None, :])
    return final


# revision 54
# speedup vs baseline: 1.0002x; 1.0002x over previous
"""Multi-head attention (B=4, T=S=2048, E=1024, H=16, D=64) on 8 TRN2 NeuronCores.

Sharding: core c handles batch b=c//2 and head-group g=c%2 (8 of 16 heads).
Each core computes its 8 heads' attention plus the matching column-slice of
the output projection, producing a partial [T, E] f32 output. Host sums the
two partials per batch and adds bo.

On-chip dataflow (fp32 PSUM accumulation throughout):
  qT[d,t] = WqT.T @ queryT       (d-major projections, per 128-dim head pair;
                                  fp8 DoubleRow with hi+lo error compensation:
                                  xh*Wh + xl*Wh + xh*Wl at 0.5 cycles/row)
  kT[d,t] likewise; v[s,d] natural via value.T as the stationary operand
  S.T[s,t] = kT_h.T @ qT_h       (two heads row-packed in the 128-row PE array)
  expS.T   = exp(S.T * 1/8)      (ScalarE, PSUM -> SBUF bf16)
  [O;den]  = expS_blk.T @ [v_h|1]  (exp block stationary, narrow 65-col v
                                    moving -> O in [t,d] layout + per-t dens)
  Onorm_td = O * (1/den)         (den is per-PARTITION here: one cheap DVE
                                  tensor_tensor, no cross-partition bcast)
  OnormT   = XBAR DMA-transpose of Onorm_td -> d-major   (no PE/PSUM cost)
  partial  = OnormT.T @ WoSlice  (accumulate over the core's 4 head pairs)

The [t,d]-output PV orientation keeps the PE's moving operand narrow (65
cols vs 512), halving PV time; denominators ride along as a 65th moving
column of ones, and the d-major layout the out-projection needs is restored
by the DMA crossbar instead of the PE.

Activations and projection weights arrive as host-packed fp8 hi/lo pair
layouts (pre-scaled into e4m3's normal range; the 2^-15 product scale is
folded into the f32 bias-add), streamed in few large DMAs issued a stage
ahead of their consumers; projections are emitted as quarter-granular
accumulation groups so their PSUM residency is short and their PE work
spreads smoothly across the score slots.

Emission is software-pipelined: stage s=(pair, t-quarter) in pair-major
order; each stage's 16 score-tile slots interleave the previous stage's PV
accumulation plus spread-out projection / v-projection / out-projection
work, keeping ScalarE (the exp bottleneck) continuously fed.
"""

from contextlib import ExitStack

import numpy as np
import ml_dtypes

B, T, S, E = 4, 2048, 2048, 1024
H, D = 16, 64
DC = 512          # dims per core (8 heads x 64)
NP = 4            # head pairs per core
NS = S // 128     # 16 s-tiles
NQ = 4            # t-quarters of 512

_BF16 = ml_dtypes.bfloat16
_F8 = ml_dtypes.float8_e4m3fn


def _pair_planes(xT, scale):
    """Split into fp8 hi+lo planes after pre-scaling into e4m3's normal
    range (the raw values sit near the subnormal floor, where the lo plane
    would quantize away); the matmul results carry the combined scale,
    removed in the f32 bias-add."""
    xs = xT.astype(np.float32) * scale
    hi = xs.astype(_F8)
    lo = (xs - hi.astype(np.float32)).astype(_F8)
    return hi, lo


def _x8_layout(xT):
    """[E, 2048] f32 -> [128, 32768] fp8 hi/lo chunk-pair layout: col =
    q*8192 + P*2048 + c2*1024 + pl*512 + tau, row p, where the source row
    is e = (2P + c2)*128 + p and t = q*512 + tau."""
    hi, lo = _pair_planes(xT, 32.0)
    A = np.stack([hi, lo]).reshape(2, 4, 2, 128, 4, 512)
    A = A.transpose(3, 4, 1, 2, 0, 5)
    return np.ascontiguousarray(A.reshape(128, 4 * 8192))


def _w8_layout(WT):
    """[E, DC] f32 -> [128, 8192] fp8: col = P*2048 + c2*1024 + pl*512 + d."""
    hi, lo = _pair_planes(WT, 1024.0)
    A = np.stack([hi, lo]).reshape(2, 4, 2, 128, 512)
    A = A.transpose(3, 1, 2, 0, 4)
    return np.ascontiguousarray(A.reshape(128, 8192))

_cached = None


def _build(repeats=1):
    import concourse.bass as bass
    import concourse.mybir as mybir
    import concourse.tile as tile
    from concourse import bacc

    f32 = mybir.dt.float32
    bf16 = mybir.dt.bfloat16
    i32 = mybir.dt.int32
    AF = mybir.ActivationFunctionType
    # Schraudolph exp constants (y in units of raw score; exp(y/8) =
    # bitcast_f32(int32(y * 2^23/(8 ln2) + (127*2^23 - C)))), C tuned for
    # min RMS relative error under truncating conversion.
    SCH_A = float(np.float32(2 ** 23 / np.log(2) / 8.0))
    SCH_B = float(np.float32(127 * 2 ** 23 - 488000))

    nc = bacc.Bacc("TRN2", target_bir_lowering=False)

    f8 = mybir.dt.float8e4
    # fp8 hi/lo pair layouts (host-packed to match SBUF tiles exactly):
    # activations [128, 4 quarters x 8192] with quarter cols
    # P*2048 + c2*1024 + pl*512 + tau  (chunk-pair P, chunk c2, hi/lo pl);
    # weights [128, 8192] with cols P*2048 + c2*1024 + pl*512 + d.
    q8_d = nc.dram_tensor("q8", [128, 4 * 8192], f8, kind="ExternalInput")
    k8_d = nc.dram_tensor("k8", [128, 4 * 8192], f8, kind="ExternalInput")
    v8_d = nc.dram_tensor("v8", [128, 4 * 8192], f8, kind="ExternalInput")
    wq8_d = nc.dram_tensor("wq8", [128, 8192], f8, kind="ExternalInput")
    wk8_d = nc.dram_tensor("wk8", [128, 8192], f8, kind="ExternalInput")
    wv8_d = nc.dram_tensor("wv8", [128, 8192], f8, kind="ExternalInput")
    WoS_d = nc.dram_tensor("WoS", [DC, E], bf16, kind="ExternalInput")
    bq_d = nc.dram_tensor("bq", [128, NP], f32, kind="ExternalInput")
    bk_d = nc.dram_tensor("bk", [128, NP], f32, kind="ExternalInput")
    bv_d = nc.dram_tensor("bv", [1, DC], f32, kind="ExternalInput")
    out_d = nc.dram_tensor("out", [T, E], bf16, kind="ExternalOutput")

    with tile.TileContext(nc) as tc, ExitStack() as ctx:
        persist = ctx.enter_context(tc.tile_pool(name="persist", bufs=1))
        psc = ctx.enter_context(tc.tile_pool(name="psc", bufs=2, space="PSUM"))
        ppv = ctx.enter_context(tc.tile_pool(name="ppv", bufs=1, space="PSUM"))
        pmx = ctx.enter_context(tc.tile_pool(name="pmx", bufs=2, space="PSUM"))
        expool = ctx.enter_context(tc.tile_pool(name="expool", bufs=22))
        small = ctx.enter_context(tc.tile_pool(name="small", bufs=2))
        ocp_pool = ctx.enter_context(tc.tile_pool(name="ocp", bufs=2))
        xin = ctx.enter_context(tc.tile_pool(name="xin", bufs=14))
        wpool = ctx.enter_context(tc.tile_pool(name="wts", bufs=3))

        # ---- persistent SBUF tiles ----
        qTs = [persist.tile([128, T], bf16, tag=f"qT{p}", name=f"qT{p}") for p in range(NP)]
        kTs = [persist.tile([128, S], bf16, tag=f"kT{p}", name=f"kT{p}") for p in range(NP)]
        vaug = [persist.tile([128, 8 * 65], bf16, tag=f"va{st}", name=f"va{st}") for st in range(NS)]
        WoSs = [persist.tile([128, E], bf16, tag=f"wo{p}", name=f"wo{p}") for p in range(NP)]
        Onorm = [persist.tile([128, T], bf16, tag=f"on{p}", name=f"on{p}") for p in range(NP)]
        bq_sb = persist.tile([128, NP], f32, tag="bq", name="bq_sb")
        bk_sb = persist.tile([128, NP], f32, tag="bk", name="bk_sb")
        bv_sb = persist.tile([128, DC], f32, tag="bv", name="bv_sb")

        def load_biases():
            nc.sync.dma_start(out=bq_sb, in_=bq_d[:, :])
            nc.sync.dma_start(out=bk_sb, in_=bk_d[:, :])
            bv_ap = bv_d[:, :]
            bv_bcast_ap = bass.AP(
                tensor=bv_ap.tensor,
                offset=bv_ap.offset,
                ap=[[0, 128], bv_ap.ap[-1]],
            )
            nc.sync.dma_start(out=bv_sb, in_=bv_bcast_ap)
        for st in range(NS):
            va3 = vaug[st].rearrange("p (h x) -> p h x", x=65)
            nc.vector.memset(va3[:, :, 64:65], 1.0)

        def load_w(dram, split=False):
            """One [128, 8192] fp8 tile holding the hi/lo chunk-pair layout
            of one projection weight (single DMA). With split=True, pair-0's
            128-col d-slices (the only ones the startup stages touch) load
            first and a thunk for the rest is returned."""
            wt = wpool.tile([128, 8192], f8, tag="w", name="wt")
            w4 = wt.rearrange("p (b d) -> p b d", d=512)
            s4 = dram[:, :].rearrange("p (b d) -> p b d", d=512)
            if not split:
                nc.sync.dma_start(out=wt, in_=dram[:, :])
                return wt
            nc.sync.dma_start(out=w4[:, :, 0:128], in_=s4[:, :, 0:128])
            def rest():
                nc.sync.dma_start(out=w4[:, :, 128:512], in_=s4[:, :, 128:512])
            return wt, rest

        def pair_ap(t, off, n):
            """AP selecting the two 1024-stride k-tile blocks at off: free
            dims [2, n] as DoubleRow wants."""
            a = t[:, off:off + n]
            return bass.AP(tensor=a.tensor, offset=a.offset,
                           ap=[a.ap[0], [1024, 2], [1, n]])

        def xq_dma(x_dram, q):
            """Two [128, 2x2048] fp8 tiles (chunk-pairs 0-1 and 2-3) of one
            512-col quarter of an activation input (two contiguous DMAs)."""
            xh = []
            for hv in range(2):
                xt = xin.tile([128, 4096], f8, tag="xin", name="xin")
                nc.sync.dma_start(
                    out=xt,
                    in_=x_dram[:, q * 8192 + hv * 4096:
                               q * 8192 + (hv + 1) * 4096],
                )
                xh.append(xt)
            return xh

        def dma_box(x_dram, q):
            """Thunk that issues xq_dma when called (placed a stage ahead
            of the matmul consumers), exposing the tiles via .get()."""
            box = []
            def run():
                box.extend(xq_dma(x_dram, q))
            run.get = lambda: box
            return run

        def proj_quarter(p, wt, get_xq, dst, bias_sb, q):
            """Work thunks for one projection output quarter: a 2-thunk,
            12-matmul fp8 DoubleRow accumulation group computing the
            error-compensated xh*Wh + xl*Wh + xh*Wl over all 4 chunk-pairs,
            then bias. get_xq resolves the quarter's input tiles at emission
            time (they were DMA'd a stage earlier)."""
            ps_box = []
            DR = mybir.MatmulPerfMode.DoubleRow

            def half(P0, first, last, ps_box=ps_box):
                xh = get_xq()
                if first:
                    ps_box.append(pmx.tile([128, 512], f32, tag="mx", name="mx_ps"))
                for P in (P0, P0 + 1):
                    loc = (P % 2) * 2048
                    wh = P * 2048 + p * 128
                    for i, (s_off, m_off) in enumerate(
                            ((wh, loc), (wh, loc + 512), (wh + 512, loc))):
                        nc.tensor.matmul(
                            ps_box[0],
                            pair_ap(wt, s_off, 128),
                            pair_ap(xh[P // 2], m_off, 512),
                            start=(first and P == P0 and i == 0),
                            stop=(last and P == P0 + 1 and i == 2),
                            perf_mode=DR,
                        )

            def mm_lo():
                half(0, True, False)

            def mm_hi(ps_box=ps_box):
                half(2, False, True)
                nc.vector.tensor_scalar(
                    out=dst[:, q * 512:(q + 1) * 512],
                    in0=ps_box[0], scalar1=1.0 / (32.0 * 1024.0),
                    scalar2=bias_sb[:, p:p + 1],
                    op0=mybir.AluOpType.mult, op1=mybir.AluOpType.add)
                ps_box.clear()

            return [mm_lo, mm_hi]

        def vproj_units(wt, dh, get_vq, stis):
            """V-projection work for head-quad dh: per unit the
            error-compensated fp8 DoubleRow full-E contraction for one
            128-row s-tile + bias into the augmented-v layout. get_vq(st//4)
            resolves the s-quarter's input tiles."""
            thunks = []
            DR = mybir.MatmulPerfMode.DoubleRow
            for st in stis:
                def vst(st=st):
                    vh = get_vq(st // 4)
                    si = st % 4
                    ps = pmx.tile([128, 512], f32, tag="mx", name="mx_ps")
                    for P in range(4):
                        loc = (P % 2) * 2048 + si * 128
                        wo = P * 2048 + dh * 256
                        for i, (s_off, m_off) in enumerate(
                                ((loc, wo), (loc + 512, wo), (loc, wo + 512))):
                            nc.tensor.matmul(
                                ps[:, 0:256],
                                pair_ap(vh[P // 2], s_off, 128),
                                pair_ap(wt, m_off, 256),
                                start=(P == 0 and i == 0),
                                stop=(P == 3 and i == 2),
                                perf_mode=DR,
                            )
                    va3 = vaug[st].rearrange("p (h x) -> p h x", x=65)
                    nc.vector.scalar_tensor_tensor(
                        out=va3[:, dh * 4:(dh + 1) * 4, 0:64],
                        in0=ps[:, 0:256].rearrange("p (h x) -> p h x", x=64),
                        scalar=1.0 / (32.0 * 1024.0),
                        in1=bv_sb[:, dh * 256:(dh + 1) * 256].rearrange(
                            "p (h x) -> p h x", x=64),
                        op0=mybir.AluOpType.mult, op1=mybir.AluOpType.add,
                    )
                thunks.append(vst)
            return thunks

        def outproj_thunks(tq):
            """8 [t-block, 512]-column units. The first two are split into
            open-with-pairs-0-2 / append-pair-3 thunk pairs: pair 3's Onorm
            quarter arrives via the freshest transpose, so priming both PSUM
            groups first hides that latency from the in-order PE queue."""
            units = [(tt, c) for tt in range(tq * 4, tq * 4 + 4)
                     for c in range(2)]
            boxes = {}

            def emit_mms(ps, tt, c, prange, start, stop):
                for p in prange:
                    nc.tensor.matmul(
                        ps,
                        Onorm[p][:, tt * 128:(tt + 1) * 128],
                        WoSs[p][:, c * 512:(c + 1) * 512],
                        start=(start and p == prange[0]),
                        stop=(stop and p == prange[-1]),
                    )

            def store(ps, tt, c):
                oc = ocp_pool.tile([128, 512], bf16, tag="ocp", name="oc",
                                   bufs=3)
                nc.vector.tensor_copy(oc, ps)
                nc.sync.dma_start(
                    out=out_d[tt * 128:(tt + 1) * 128,
                              c * 512:(c + 1) * 512],
                    in_=oc)

            def pre(i):
                def f(i=i):
                    tt, c = units[i]
                    ps = pmx.tile([128, 512], f32, tag="mx", name="mx_ps")
                    boxes[i] = ps
                    emit_mms(ps, tt, c, (0, 1, 2), True, False)
                return f

            def fin(i):
                def f(i=i):
                    tt, c = units[i]
                    ps = boxes.pop(i)
                    emit_mms(ps, tt, c, (3,), False, True)
                    store(ps, tt, c)
                return f

            def unit(i):
                def f(i=i):
                    tt, c = units[i]
                    ps = pmx.tile([128, 512], f32, tag="mx", name="mx_ps")
                    emit_mms(ps, tt, c, (0, 1, 2, 3), True, True)
                    store(ps, tt, c)
                return f

            return ([pre(0), pre(1), fin(0), fin(1)]
                    + [unit(i) for i in range(2, 8)])

        def outproj_tail(tq, tts=None):
            # tail variant: "sc" psum tiles are free once scoring has ended,
            # so use wide [128,1024] units to avoid mx-slot serialization
            def run():
                for tt in (tts if tts is not None
                           else range(tq * 4, tq * 4 + 4)):
                    op_ps = psc.tile([128, 1024], f32, tag="sc", name="sc_ps")
                    for c in range(2):
                        for p in range(NP):
                            nc.tensor.matmul(
                                op_ps[:, c * 512:(c + 1) * 512],
                                Onorm[p][:, tt * 128:(tt + 1) * 128],
                                WoSs[p][:, c * 512:(c + 1) * 512],
                                start=(p == 0),
                                stop=(p == 3),
                            )
                    oc = ocp_pool.tile([128, 1024], bf16, tag="ocpw", name="ocw")
                    nc.vector.tensor_copy(oc, op_ps)
                    nc.sync.dma_start(out=out_d[tt * 128:(tt + 1) * 128, :],
                                      in_=oc)
            return run

        class PrevStage:
            def __init__(self, p, tq, exs):
                self.p, self.tq, self.exs = p, tq, exs
                self.pv = None      # two [128, 4*65] psum tiles (4 units each)

        def emit_pv_slot(prev, st):
            """8 accumulating matmuls: O_td[t, d]/den for each (tb, h) unit,
            with exp (s x t block) stationary and the 65-col augmented v
            moving. Unit g = tb*2 + h lives at cols (g%4)*65 of pv tile g//4;
            one start per PSUM bank (unit 0 of each tile, which pending-zeroes
            the whole bank), one stop (unit 3)."""
            if st == 0:
                prev.pv = [
                    ppv.tile([128, 260], f32, tag="pvA", name="pvA",
                             padded_shape=[128, 512]),
                    ppv.tile([128, 260], f32, tag="pvB", name="pvB",
                             padded_shape=[128, 512]),
                ]
            for g in range(8):
                tb, h = g // 2, g % 2
                pvt = prev.pv[g // 4]
                u = g % 4
                hidx = 2 * prev.p + h
                nc.tensor.matmul(
                    pvt[:, u * 65:(u + 1) * 65],
                    prev.exs[st][:, h * 512 + tb * 128:h * 512 + (tb + 1) * 128],
                    vaug[st][:, hidx * 65:hidx * 65 + 65],
                    start=(st == 0 and u == 0),
                    stop=(st == 15 and u == 3),
                )

        def emit_normalize_half(prev, i, rc, otd):
            """Normalize + transpose for one pv tile (4 units = 2 t-blocks):
            used at the tail so the first out-proj blocks start while the
            second half still normalizes."""
            pv3 = prev.pv[i].rearrange("p (u x) -> p u x", x=65)
            nc.vector.reciprocal(rc[:, i * 4:(i + 1) * 4], pv3[:, :, 64:65])
            rc_b = bass.AP(
                tensor=rc.tensor,
                offset=rc[:, i * 4:(i + 1) * 4].offset,
                ap=[rc.ap[0], [1, 4], [0, 64]],
            )
            nc.vector.tensor_mul(
                otd[:, i * 256:(i + 1) * 256].rearrange(
                    "p (u x) -> p u x", x=64),
                pv3[:, :, 0:64],
                rc_b,
            )
            t0 = prev.tq * 512 + i * 256
            nc.sync.dma_start_transpose(
                out=Onorm[prev.p][:, t0:t0 + 256].rearrange(
                    "p (b t) -> p b t", t=128),
                in_=otd[:, i * 256:(i + 1) * 256],
            )

        def emit_normalize(prev):
            """den sits on the same partition as its O row: reciprocal of the
            65th column of each unit, one tensor_tensor mul per pv tile (rc
            broadcast along d via a stride-0 AP) into a [t, d] bf16 tile, then
            one XBAR DMA transpose restoring the d-major layout out-proj
            wants (out 3D => per-128-block transpose)."""
            rc = small.tile([128, 8], f32, tag="rc", name="rc")
            otd = small.tile([128, 512], bf16, tag="otd", name="otd")
            for i in range(2):
                pv3 = prev.pv[i].rearrange("p (u x) -> p u x", x=65)
                nc.vector.reciprocal(rc[:, i * 4:(i + 1) * 4], pv3[:, :, 64:65])
            for i in range(2):
                pv3 = prev.pv[i].rearrange("p (u x) -> p u x", x=65)
                rc_b = bass.AP(
                    tensor=rc.tensor,
                    offset=rc[:, i * 4:(i + 1) * 4].offset,
                    ap=[rc.ap[0], [1, 4], [0, 64]],
                )
                nc.vector.tensor_mul(
                    otd[:, i * 256:(i + 1) * 256].rearrange(
                        "p (u x) -> p u x", x=64),
                    pv3[:, :, 0:64],
                    rc_b,
                )
            t0 = prev.tq * 512
            nc.sync.dma_start_transpose(
                out=Onorm[prev.p][:, t0:t0 + 512].rearrange(
                    "p (b t) -> p b t", t=128),
                in_=otd,
            )
            prev.pv = None

        DVE_EXP_ST = (4, 9, 14)

        def emit_stage(p, tq, prev, extras, dl=16, sl0=0, dve_exp=True):
            """16 score slots for (p, tq); interleave prev stage's PV and
            the extra thunks (emitted between slots sl0..sl0+dl); returns
            this stage's record. Three slots per stage compute exp on the
            DVE (Schraudolph bit-trick, ~1.8% rms) to keep ScalarE off the
            critical path; the rest use the exact ScalarE table exp."""
            t0 = tq * 512
            exs = []
            n_ex = len(extras)
            taken = 0
            for st in range(NS):
                # scores lead the slot: exp(st) can then start while the PE
                # works through this slot's PV and extras, keeping ScalarE
                # saturated; the score psum's WAR wait (on exp st-2) is
                # covered by the previous slot's trailing work.
                sc_ps = psc.tile([128, 1024], f32, tag="sc", name="sc_ps")
                nc.tensor.matmul(
                    sc_ps[:, 0:512],
                    kTs[p][0:64, st * 128:(st + 1) * 128],
                    qTs[p][0:64, t0:t0 + 512],
                    start=True, stop=True,
                    tile_position=(0, 0),
                )
                nc.tensor.matmul(
                    sc_ps[:, 512:1024],
                    kTs[p][64:128, st * 128:(st + 1) * 128],
                    qTs[p][64:128, t0:t0 + 512],
                    start=True, stop=True,
                    tile_position=(64, 0),
                )
                ex = expool.tile([128, 1024], bf16, tag="ex", name="ex")
                if dve_exp and st in DVE_EXP_ST:
                    sch = small.tile([128, 1024], i32, tag="sch", name="sch",
                                     bufs=1)
                    nc.vector.tensor_scalar(
                        out=sch, in0=sc_ps, scalar1=SCH_A, scalar2=SCH_B,
                        op0=mybir.AluOpType.mult, op1=mybir.AluOpType.add)
                    nc.vector.tensor_copy(ex, sch.bitcast(f32))
                else:
                    nc.scalar.activation(ex, sc_ps, AF.Exp, scale=0.125)
                exs.append(ex)
                if prev is not None:
                    emit_pv_slot(prev, st)
                if st >= sl0:
                    prog = min(st - sl0 + 1, dl)
                    want = (n_ex * prog + dl - 1) // dl
                    while taken < want:
                        extras[taken]()
                        taken += 1
            while taken < n_ex:
                extras[taken]()
                taken += 1
            if prev is not None:
                rc = small.tile([128, 8], f32, tag="rc", name="rc")
                otd = small.tile([128, 512], bf16, tag="otd", name="otd")
                emit_normalize_half(prev, 0, rc, otd)
                emit_normalize_half(prev, 1, rc, otd)
                prev.pv = None
            return PrevStage(p, tq, exs)

        # ---- emission ----
        for _rep in range(repeats):
            # startup: stream in pair-0's q/k projections at quarter
            # granularity — the first scores need only q quarter-0 and k
            # quarter-0; later k quarters land just before their score
            # slots. Only pair-0's weight slices load up-front.
            wq, wq_rest = load_w(wq8_d, split=True)
            xq_q0 = xq_dma(q8_d, 0)
            wk, wk_rest = load_w(wk8_d, split=True)
            xk_q0 = xq_dma(k8_d, 0)
            load_biases()
            xk_q1 = xq_dma(k8_d, 1)
            for th in proj_quarter(0, wq, lambda: xq_q0, qTs[0], bq_sb, 0):
                th()
            for th in proj_quarter(0, wk, lambda: xk_q0, kTs[0], bk_sb, 0):
                th()
            xk_q2 = xq_dma(k8_d, 2)
            xq_q1 = xq_dma(q8_d, 1)            # q quarter 1: stage 1's scores
            xk_q3 = xq_dma(k8_d, 3)
            for th in proj_quarter(0, wk, lambda: xk_q1, kTs[0], bk_sb, 1):
                th()
            wk_rest()
            wq_rest()
            wv = load_w(wv8_d)
            for p in range(NP):
                nc.sync.dma_start(out=WoSs[p], in_=WoS_d[p * 128:(p + 1) * 128, :])

            extras = {}

            def add(sg, ths):
                extras[sg] = extras.get(sg, []) + ths

            # stage 0: k quarters 2/3 (this stage's slots 8-15), q quarter 1
            # (stage 1's scores), vaug st0-7; stage 1: q quarters 2/3
            # (stage 2's scores), vaug st8-15.
            xv_q0_t = dma_box(v8_d, 0)
            xv_q1_t = dma_box(v8_d, 1)
            xv_q2_t = dma_box(v8_d, 2)
            xv_q3_t = dma_box(v8_d, 3)
            xq_q2_t = dma_box(q8_d, 2)
            xq_q3_t = dma_box(q8_d, 3)

            def get_vA(qv):
                return {0: xv_q0_t.get, 1: xv_q1_t.get,
                        2: xv_q2_t.get, 3: xv_q3_t.get}[qv]()

            add(0, [xv_q0_t, xv_q1_t])
            add(0, proj_quarter(0, wk, lambda: xk_q2, kTs[0], bk_sb, 2))
            add(0, proj_quarter(0, wk, lambda: xk_q3, kTs[0], bk_sb, 3))
            add(0, proj_quarter(0, wq, lambda: xq_q1, qTs[0], bq_sb, 1))
            add(0, [xv_q2_t, xv_q3_t, xq_q2_t, xq_q3_t])
            add(0, vproj_units(wv, 0, get_vA, range(0, 8)))
            add(1, proj_quarter(0, wq, xq_q2_t.get, qTs[0], bq_sb, 2))
            add(1, proj_quarter(0, wq, xq_q3_t.get, qTs[0], bq_sb, 3))
            add(1, vproj_units(wv, 0, get_vA, range(8, 16)))

            # pairs 1-3 q/k at quarter granularity: k q0/q1 at stage 4p-2,
            # q q0/q1 at 4p-1, k q2/q3 at 4p (hard slot-8 deadline), q q2/q3
            # at 4p+1; each quarter's two DMAs issue one stage ahead.
            for p in range(1, NP):
                bx = {key: dma_box(d, q)
                      for key, d, q in (("k0", k8_d, 0), ("k1", k8_d, 1),
                                        ("k2", k8_d, 2), ("k3", k8_d, 3),
                                        ("q0", q8_d, 0), ("q1", q8_d, 1),
                                        ("q2", q8_d, 2), ("q3", q8_d, 3))}
                add(4 * p - 3, [bx["k0"], bx["k1"]])
                add(4 * p - 2, proj_quarter(p, wk, bx["k0"].get, kTs[p], bk_sb, 0)
                    + proj_quarter(p, wk, bx["k1"].get, kTs[p], bk_sb, 1)
                    + [bx["q0"], bx["q1"]])
                add(4 * p - 1, proj_quarter(p, wq, bx["q0"].get, qTs[p], bq_sb, 0)
                    + proj_quarter(p, wq, bx["q1"].get, qTs[p], bq_sb, 1)
                    + [bx["k2"], bx["k3"]])
                add(4 * p, proj_quarter(p, wk, bx["k2"].get, kTs[p], bk_sb, 2)
                    + proj_quarter(p, wk, bx["k3"].get, kTs[p], bk_sb, 3)
                    + [bx["q2"], bx["q3"]])
                add(4 * p + 1, proj_quarter(p, wq, bx["q2"].get, qTs[p], bq_sb, 2)
                    + proj_quarter(p, wq, bx["q3"].get, qTs[p], bq_sb, 3))

            # second head-quad of V (pairs 2-3, needed from stage 9's PV):
            # spread over lighter stages 2/3/5/6, s-quarter DMAs one stage
            # ahead (the vT data is re-fetched; keeping the vpA tiles alive
            # that long would overflow the input pool).
            vB = {qv: dma_box(v8_d, qv) for qv in range(4)}

            def get_vB(qv):
                return vB[qv].get()

            add(1, [vB[0]])
            add(2, vproj_units(wv, 1, get_vB, range(0, 4)) + [vB[1]])
            add(3, vproj_units(wv, 1, get_vB, range(4, 8)))
            add(4, [vB[2]])
            add(5, vproj_units(wv, 1, get_vB, range(8, 12)) + [vB[3]])
            add(6, vproj_units(wv, 1, get_vB, range(12, 16)))

            add(14, outproj_thunks(0))
            add(15, outproj_thunks(1))

            # DMA-issue thunks cost nothing on the PE: run them at each
            # stage's first slots so their transfers lead the consumers.
            for sg in extras:
                extras[sg].sort(key=lambda th: not hasattr(th, "get"))

            # pacing deadlines: K-half1 stages (4p) must finish extras by
            # slot 8 (their own scores need those kT columns); stages 0/1
            # feed vaug just-in-time; elsewhere spread smoothly. Stages
            # 14/15's out-proj starts at slot 2 (their Onorm quarter lands
            # via the DMA transpose issued at the previous stage's end).
            dls = {0: 16, 1: 14, 4: 8, 8: 8, 12: 8, 14: 15, 15: 15}
            sl0s = {14: 4, 15: 4}
            prev = None
            for s in range(16):
                p, tq = s // 4, s % 4
                prev = emit_stage(p, tq, prev, extras.get(s, []),
                                  dl=dls.get(s, 16), sl0=sl0s.get(s, 0),
                                  dve_exp=False)

            # tail: PV of the last stage with out-proj(t2) interleaved
            # (its Onorm slices land with the transpose at the end of stage
            # 15), then the final normalize+transpose and out-proj(t3)
            op2 = outproj_thunks(2)
            taken = 0
            for st in range(NS):
                if st >= 3:
                    want = (len(op2) * (st - 2) + 12) // 13
                    while taken < want:
                        op2[taken]()
                        taken += 1
                emit_pv_slot(prev, st)
            # pre-accumulate pairs 0-2 of out-proj(q3) for t-blocks 12/13
            # (their Onorm quarters landed at stages 4/8/12) while pair 3's
            # normalize -> XBAR-transpose chain drains; append only the p3
            # matmuls once it lands.
            pre = []
            for tt in (12, 13):
                op_ps = psc.tile([128, 1024], f32, tag="sc", name="sc_ps")
                for c in range(2):
                    for p in range(3):
                        nc.tensor.matmul(
                            op_ps[:, c * 512:(c + 1) * 512],
                            Onorm[p][:, tt * 128:(tt + 1) * 128],
                            WoSs[p][:, c * 512:(c + 1) * 512],
                            start=(p == 0), stop=False,
                        )
                pre.append((tt, op_ps))
            pre2 = []
            for c in range(2):       # t-block 14's halves prime the mx slots
                ps = pmx.tile([128, 512], f32, tag="mx", name="mx_ps")
                for p in range(3):
                    nc.tensor.matmul(
                        ps, Onorm[p][:, 14 * 128:15 * 128],
                        WoSs[p][:, c * 512:(c + 1) * 512],
                        start=(p == 0), stop=False,
                    )
                pre2.append(ps)
            rc = small.tile([128, 8], f32, tag="rc", name="rc")
            otd = small.tile([128, 512], bf16, tag="otd", name="otd")
            emit_normalize_half(prev, 0, rc, otd)
            emit_normalize_half(prev, 1, rc, otd)
            for tt, op_ps in pre:
                for c in range(2):
                    nc.tensor.matmul(
                        op_ps[:, c * 512:(c + 1) * 512],
                        Onorm[3][:, tt * 128:(tt + 1) * 128],
                        WoSs[3][:, c * 512:(c + 1) * 512],
                        start=False, stop=(c == 1),
                    )
                oc = ocp_pool.tile([128, 1024], bf16, tag="ocpw", name="ocw")
                nc.vector.tensor_copy(oc, op_ps)
                nc.sync.dma_start(out=out_d[tt * 128:(tt + 1) * 128, :], in_=oc)
            for c in range(2):
                ps = pre2[c]
                nc.tensor.matmul(
                    ps, Onorm[3][:, 14 * 128:15 * 128],
                    WoSs[3][:, c * 512:(c + 1) * 512],
                    start=False, stop=True,
                )
                oc = ocp_pool.tile([128, 512], bf16, tag="ocp", name="oc",
                                   bufs=3)
                nc.vector.tensor_copy(oc, ps)
                nc.sync.dma_start(
                    out=out_d[14 * 128:15 * 128, c * 512:(c + 1) * 512],
                    in_=oc)
            outproj_tail(3, (15,))()
            prev.pv = None

    nc.compile()
    return nc


def _get_nc():
    global _cached
    if _cached is None:
        _cached = _build()
    return _cached


def _prep_core_inputs(c, query, key, value, Wq, Wk, Wv, Wo, bq, bk, bv,
                      _cache={}):
    b, g = c // 2, c % 2
    sl = slice(g * DC, (g + 1) * DC)
    key_ = (id(query), b)
    if key_ not in _cache:
        # both cores of a batch share the packed fp8 activations
        _cache.clear()
        _cache[key_] = {
            "q8": _x8_layout(query[b].T),
            "k8": _x8_layout(key[b].T),
            "v8": _x8_layout(value[b].T),
        }
    shared = _cache[key_]
    return {
        **shared,
        "wq8": _w8_layout(Wq[sl].T),
        "wk8": _w8_layout(Wk[sl].T),
        "wv8": _w8_layout(Wv[sl].T),
        "WoS": Wo[:, sl].T.astype(_BF16),
        "bq": np.ascontiguousarray(bq[sl].reshape(NP, 128).T),
        "bk": np.ascontiguousarray(bk[sl].reshape(NP, 128).T),
        "bv": np.ascontiguousarray(bv[sl].reshape(1, DC)),
    }


def kernel(**inputs):
    from concourse.bass_utils import run_bass_kernel_spmd

    args = {k: np.asarray(inputs[k], np.float32)
            for k in ("query", "key", "value", "Wq", "Wk", "Wv", "Wo",
                      "bq", "bk", "bv", "bo")}
    _prep_core_inputs.__defaults__[0].clear()
    nc = _get_nc()
    in_maps = [
        _prep_core_inputs(c, args["query"], args["key"], args["value"],
                          args["Wq"], args["Wk"], args["Wv"], args["Wo"],
                          args["bq"], args["bk"], args["bv"])
        for c in range(8)
    ]
    res = run_bass_kernel_spmd(nc, in_maps, core_ids=list(range(8)))
    outs = [r["out"] for r in res.results]
    final = np.empty((B, T, E), np.float32)
    for b in range(B):
        final[b] = (outs[2 * b].astype(np.float32)
                    + outs[2 * b + 1].astype(np.float32)
                    + args["bo"][None, :])
    return final
